# revision 29
# baseline (speedup 1.0000x reference)
"""LiteLinear (dense linear + routed LoRA) Trainium2 kernel.

out = x @ W^T + bias + scaling[aid] * ((x @ la[aid]^T) @ lb[aid]^T)   (aid>0)

Data-parallel over tokens (16384 -> 2048/core on 8 cores); weights and LoRA
stacks replicated. Output computed transposed ([d_out, tok]) so the PSUM
eviction is one fused scalar-engine op (psum*scale + bias[partition]).

Numerics: the big matmul runs in fp8 e4m3 with perf_mode=DoubleRow (2 k-planes
per instruction), W pre-scaled by 32 so its values sit in e4m3's normal range;
the 1/32 descale folds into the eviction. The LoRA path (u = x @ la^T, masked,
then delta = u_m @ lb^T) stays bf16: fp8 there doubles the delta error and
blows the error budget. lb carries scaling*32 so delta accumulates into the
same scaled PSUM. Measured end-to-end max rel err ~1e-2 (gate 2e-2).

Per core SBUF: xtb [128,16,2048] bf16 (x^T, k-chunked), xf8 same in e4m3
(device-side DVE cast), wf8 [128,256,128] e4m3 (host-packed so per-column
DMAs land contiguous), lat/lbt/selt bf16, bias [128,16] f32.

Schedule (timeline-sim tuned, ~95.7us vs 291us bf16 baseline):
- PE warmup matmuls on memset tiles from t~0 ride out the p-state ramp.
- x streams in token blocks (t0 at k-pair granularity, t1-3 in 4-chunk
  quarters); DVE casts + u-matmuls chase the stream; wf8 column batches and
  per-block selt slices interleave at tuned points.
- main tiles (n,t): 8 DoubleRow MMs + 1 bf16 LoRA MM into one PSUM bank
  (opool bufs=6 rotation), evicted via ACT (scale+bias, fp16) into per-run
  staging tiles, flushed as one DMA per 4-column run.
- out-flushes for waves that overlap the in-stream are DEFERRED until the
  in-stream is fully queued (out transfers otherwise steal serial DMA slots
  from the critical x/w chunks); the final wave flushes per-tile from the
  then-idle SP queue to shorten the tail.
Engine usage: PE 89% busy; evictions on ACT; casts+masks on DVE; in-DMAs
issued from SP, mid-kernel out-DMAs from ACT (avoids SP head-of-line
blocking behind eviction-dependent waits).
"""

import numpy as np
import ml_dtypes

import concourse.mybir as mybir
import concourse.tile as tile
from concourse import bacc
from concourse.bass import ts
from concourse.bass_utils import run_bass_kernel_spmd

N_CORES = 8
B, S, D_IN, D_OUT = 4, 4096, 2048, 2048
N_TOK = B * S              # 16384
TOK = N_TOK // N_CORES     # 2048 tokens per core
A, R = 8, 16
AR = A * R                 # 128
P = 128
KC = D_IN // P             # 16 contraction chunks of 128
KP = KC // 2               # 8 DoubleRow k-pairs
NT = TOK // 512            # 4 token blocks of 512
NO = D_OUT // P            # 16 d_out columns of 128
WSCALE = 32.0              # fp8 pre-scale for W (and lbt); descaled on evict

BF16 = mybir.dt.bfloat16
F16 = mybir.dt.float16
F32 = mybir.dt.float32
F8 = mybir.dt.float8e4
DR = mybir.MatmulPerfMode.DoubleRow
IDENT = mybir.ActivationFunctionType.Identity

_cached_nc = None


def _build():
    nc = bacc.Bacc("TRN2", target_bir_lowering=False, debug=False)
    xt = nc.dram_tensor("xt", [D_IN, TOK], BF16, kind="ExternalInput").ap()
    wf8 = nc.dram_tensor("wf8", [P, NO * KC * P], F8, kind="ExternalInput").ap()
    lat = nc.dram_tensor("lat", [P, KC * AR], BF16, kind="ExternalInput").ap()
    lbt = nc.dram_tensor("lbt", [AR, D_OUT], BF16, kind="ExternalInput").ap()
    selt = nc.dram_tensor("selt", [AR, TOK], BF16, kind="ExternalInput").ap()
    biasc = nc.dram_tensor("biasc", [P, NO], F32, kind="ExternalInput").ap()
    out = nc.dram_tensor("out", [D_OUT, TOK], F16, kind="ExternalOutput").ap()

    with tile.TileContext(nc) as tc:
        with (
            tc.tile_pool(name="const", bufs=1) as cpool,
            tc.tile_pool(name="work", bufs=4) as wpool,
            tc.tile_pool(name="psum_u", bufs=1, space="PSUM") as upool,
            tc.tile_pool(name="psum_o", bufs=7, space="PSUM") as opool,
        ):
            lat_sb = cpool.tile([P, KC * AR], BF16, tag="lat")
            lbt_sb = cpool.tile([P, D_OUT], BF16, tag="lbt")
            selt_sb = cpool.tile([P, TOK], BF16, tag="selt")
            bias_sb = cpool.tile([P, NO], F32, tag="bias")
            xtb_sb = cpool.tile([P, KC, TOK], BF16, tag="xtb")
            xf8_sb = cpool.tile([P, KC, TOK], F8, tag="xf8")
            wf8_sb = cpool.tile([P, NO * KC, P], F8, tag="wf8")

            # PE p-state warmup: dummy matmuls on memset tiles from t~0 so the
            # tensor engine is at full clock when the first real operand lands
            warm_a = cpool.tile([P, P], BF16, tag="warm_a")
            warm_b = cpool.tile([P, 512], BF16, tag="warm_b")
            nc.gpsimd.memset(warm_a[:], 0)
            nc.gpsimd.memset(warm_b[:], 0)
            warm_ps = upool.tile([P, 512], F32, tag="u", name="warm_ps")
            for i in range(9):
                nc.tensor.matmul(warm_ps[:], warm_a[:], warm_b[:],
                                 start=True, stop=True)

            u_m = [wpool.tile([P, 512], BF16, tag=f"um{t}", bufs=1, name=f"um{t}")
                   for t in range(NT)]

            def dma_xtb(t, k0, k1):
                nc.sync.dma_start(
                    out=xtb_sb[:, k0:k1, ts(t, 512)],
                    in_=xt[k0 * P:k1 * P, ts(t, 512)]
                    .rearrange("(k p) f -> p k f", p=P))

            def dma_selt(t):
                nc.sync.dma_start(out=selt_sb[:, ts(t, 512)],
                                  in_=selt[:, ts(t, 512)])

            def stream_xblock(t, u_ps=None, k0=0):
                # x chunks for token block t (batched 4-k-chunk DMAs) + fp8
                # cast + u accumulation
                if u_ps is None:
                    u_ps = upool.tile([P, 512], F32, tag="u", name=f"u{t}")
                for q in range(k0 // 4, KC // 4):
                    if k0 <= 4 * q:
                        dma_xtb(t, 4 * q, 4 * (q + 1))
                    nc.vector.tensor_copy(
                        out=xf8_sb[:, 4 * q:4 * (q + 1), ts(t, 512)],
                        in_=xtb_sb[:, 4 * q:4 * (q + 1), ts(t, 512)])
                    for k in range(4 * q, 4 * (q + 1)):
                        nc.tensor.matmul(
                            u_ps[:], lat_sb[:, ts(k, AR)], xtb_sb[:, k, ts(t, 512)],
                            start=(k == 0), stop=(k == KC - 1))
                nc.vector.tensor_mul(out=u_m[t][:], in0=u_ps[:],
                                     in1=selt_sb[:, ts(t, 512)])

            def dma_wcols(ns):
                n0, n1 = ns[0], ns[-1] + 1
                nc.sync.dma_start(
                    out=wf8_sb[:, n0 * KC:n1 * KC, :],
                    in_=wf8[:, n0 * KC * P:n1 * KC * P]
                    .rearrange("p (k c) -> p k c", c=P))

            deferred = []

            def flush_run(run, eng):
                t, n0, g, osb = run
                eng.dma_start(
                    out=out[n0 * P:(n0 + g) * P, ts(t, 512)]
                    .rearrange("(g p) f -> p g f", p=P),
                    in_=osb[:].rearrange("p (g f) -> p g f", g=g))

            def flush_deferred():
                for run in deferred:
                    flush_run(run, nc.sync)
                deferred.clear()

            def main_tiles(pairs, out_eng=None, defer=False):
                # pairs must be runs of contiguous n at fixed t; evictions
                # stage into one wide tile per run and flush with a single
                # DMA ([128, G, 512] SBUF -> [G*128, 512] DRAM rows), keeping
                # the out-path from fragmenting the input stream's DMA slots.
                # defer=True postpones the flush DMA until flush_deferred()
                # so out transfers don't steal DMA slots from the in-stream.
                runs = []
                for (t, n) in pairs:
                    if (runs and runs[-1][0] == t and len(runs[-1][2]) < 4
                            and runs[-1][1] + len(runs[-1][2]) == n):
                        runs[-1][2].append(n)
                    else:
                        runs.append([t, n, [n]])
                for t, n0, ns in runs:
                    g = len(ns)
                    osb = wpool.tile([P, g * 512], F16, name="osb",
                                     tag="osb" if g == 4 else "osb1", bufs=9 if g == 4 else 4)
                    for j, n in enumerate(ns):
                        o_ps = opool.tile([P, 512], F32, tag="o")
                        for kp in range(KP):
                            nc.tensor.matmul(
                                o_ps[:],
                                wf8_sb[:, n * KC + 2 * kp:n * KC + 2 * kp + 2, :],
                                xf8_sb[:, 2 * kp:2 * kp + 2, ts(t, 512)],
                                start=(kp == 0), stop=False, perf_mode=DR)
                        nc.tensor.matmul(o_ps[:], lbt_sb[:, ts(n, P)], u_m[t][:],
                                         start=False, stop=True)
                        nc.scalar.activation(osb[:, ts(j, 512)], o_ps[:], IDENT,
                                             bias=bias_sb[:, n:n + 1],
                                             scale=1.0 / WSCALE)
                    if defer:
                        deferred.append((t, n0, g, osb))
                    else:
                        flush_run((t, n0, g, osb),
                                  out_eng if out_eng is not None else nc.scalar)

            # interleave DMA stream, u path, and main tiles in operand-arrival
            # order (see module docstring timeline). Head is fine-grained so
            # the u-matmuls start as early as the DMA serialization allows.
            dma_xtb(0, 0, 2)
            nc.sync.dma_start(out=lat_sb[:], in_=lat[:, :])
            u_ps0 = upool.tile([P, 512], F32, tag="u", name="u0")
            for kp in range(1, 8):
                dma_xtb(0, 2 * kp, 2 * kp + 2)
                if kp == 5:
                    # first W columns early: main tiles (t0, n0-3) are the
                    # next PE work after u[t0] and gate on this DMA
                    dma_wcols(range(0, 4))
                nc.vector.tensor_copy(
                    out=xf8_sb[:, 2 * kp - 2:2 * kp, ts(0, 512)],
                    in_=xtb_sb[:, 2 * kp - 2:2 * kp, ts(0, 512)])
                for k in (2 * kp - 2, 2 * kp - 1):
                    nc.tensor.matmul(u_ps0[:], lat_sb[:, ts(k, AR)],
                                     xtb_sb[:, k, ts(0, 512)],
                                     start=(k == 0), stop=False)
            dma_selt(0)
            nc.vector.tensor_copy(out=xf8_sb[:, 14:16, ts(0, 512)],
                                  in_=xtb_sb[:, 14:16, ts(0, 512)])
            for k in (14, 15):
                nc.tensor.matmul(u_ps0[:], lat_sb[:, ts(k, AR)],
                                 xtb_sb[:, k, ts(0, 512)],
                                 start=False, stop=(k == 15))
            nc.vector.tensor_mul(out=u_m[0][:], in0=u_ps0[:],
                                 in1=selt_sb[:, ts(0, 512)])
            nc.sync.dma_start(out=bias_sb[:], in_=biasc[:, :])
            nc.sync.dma_start(out=lbt_sb[:], in_=lbt[:, :])
            dma_selt(1)
            main_tiles([(0, n) for n in range(0, 4)], defer=True)
            stream_xblock(1)
            dma_wcols(range(4, 8))
            dma_selt(2)
            main_tiles([(1, n) for n in range(0, 4)], defer=True)
            main_tiles([(t, n) for t in (0, 1) for n in range(4, 8)], defer=True)
            stream_xblock(2)
            dma_selt(3)
            main_tiles([(2, n) for n in range(0, 8)], defer=True)
            stream_xblock(3)
            dma_wcols(range(8, 12))
            dma_wcols(range(12, 16))
            flush_deferred()
            main_tiles([(t, n) for t in (0, 1, 2) for n in range(8, 12)])
            main_tiles([(3, n) for n in range(0, 12)])
            main_tiles([(t, n) for t in (0, 1, 2) for n in range(12, 16)],
                       out_eng=nc.sync)
            for n in range(12, 16):
                main_tiles([(3, n)], out_eng=nc.sync)
    nc.compile()
    return nc


def _get_nc():
    global _cached_nc
    if _cached_nc is None:
        _cached_nc = _build()
    return _cached_nc


def _prep_shared(weight, bias, lora_a, lora_b, scaling):
    bf16 = ml_dtypes.bfloat16
    e4 = ml_dtypes.float8_e4m3
    wt = np.asarray(weight, np.float32).T * WSCALE          # [dk, dout]
    wf8_h = np.ascontiguousarray(
        wt.reshape(KC, P, NO, P).transpose(1, 2, 0, 3).reshape(P, NO * KC * P)
    ).astype(e4)
    la = np.asarray(lora_a, np.float32).reshape(AR, D_IN)   # [ar, dk]
    lat_h = np.ascontiguousarray(
        la.reshape(AR, KC, P).transpose(2, 1, 0).reshape(P, KC * AR)
    ).astype(bf16)
    lb = np.asarray(lora_b, np.float32) * np.asarray(scaling, np.float32)[:, None, None]
    lbt_h = (np.ascontiguousarray(lb.transpose(0, 2, 1).reshape(AR, D_OUT))
             * WSCALE).astype(bf16)
    biasc_h = np.ascontiguousarray(
        np.asarray(bias, np.float32).reshape(NO, P).T)
    return wf8_h, lat_h, lbt_h, biasc_h


def _prep_core(x2, mapping, c):
    bf16 = ml_dtypes.bfloat16
    xs = x2[c * TOK:(c + 1) * TOK]
    xt_h = np.ascontiguousarray(xs.T).astype(bf16)
    ms = mapping[c * TOK:(c + 1) * TOK]
    aid = np.arange(1, A + 1, dtype=np.int32)
    onehot = (ms[None, :] == aid[:, None]).astype(np.float32)    # [A, TOK]
    selt_h = np.ascontiguousarray(np.repeat(onehot, R, axis=0)).astype(bf16)
    return xt_h, selt_h


def kernel(x, lora_mapping, weight, bias, lora_a, lora_b, scaling):
    nc = _get_nc()
    wf8_h, lat_h, lbt_h, biasc_h = _prep_shared(weight, bias, lora_a, lora_b, scaling)
    x2 = np.asarray(x, np.float32).reshape(N_TOK, D_IN)
    mapping = np.asarray(lora_mapping, np.int32)

    in_maps = []
    for c in range(N_CORES):
        xt_h, selt_h = _prep_core(x2, mapping, c)
        in_maps.append({
            "xt": xt_h, "wf8": wf8_h, "lat": lat_h, "lbt": lbt_h,
            "selt": selt_h, "biasc": biasc_h,
        })

    res = run_bass_kernel_spmd(nc, in_maps, list(range(N_CORES)))
    outs = [np.asarray(res.results[c]["out"], np.float32).T
            for c in range(N_CORES)]
    return np.ascontiguousarray(np.concatenate(outs, axis=0)).reshape(B, S, D_OUT)


# revision 50
# speedup vs baseline: 1.0127x; 1.0127x over previous
"""LiteLinear (dense linear + routed LoRA) Trainium2 kernel.

out = x @ W^T + bias + scaling[aid] * ((x @ la[aid]^T) @ lb[aid]^T)   (aid>0)

Data-parallel over tokens (16384 -> 2048/core on 8 cores); weights and LoRA
stacks replicated. Output computed transposed ([d_out, tok]) so the PSUM
eviction is one fused scalar-engine op (psum*scale + bias[partition]).

Numerics: the big matmul runs in fp8 e4m3 with perf_mode=DoubleRow (2 k-planes
per instruction), W pre-scaled by 32 so its values sit in e4m3's normal range;
the 1/32 descale folds into the eviction. The LoRA path (u = x @ la^T, masked,
then delta = u_m @ lb^T) stays bf16: fp8 there doubles the delta error and
blows the error budget. lb carries scaling*32 so delta accumulates into the
same scaled PSUM. Measured end-to-end max rel err ~1e-2 (gate 2e-2).

Per core SBUF: xtb [128,16,2048] bf16 (x^T, k-chunked), xf8 same in e4m3
(device-side DVE cast), wf8 [128,256,128] e4m3 (host-packed so per-column
DMAs land contiguous), lat/lbt/selt bf16, bias [128,16] f32.

Schedule (timeline-sim tuned, ~95.7us vs 291us bf16 baseline):
- PE warmup matmuls on memset tiles from t~0 ride out the p-state ramp.
- x streams in token blocks (t0 at k-pair granularity, t1-3 in 4-chunk
  quarters); DVE casts + u-matmuls chase the stream; wf8 column batches and
  per-block selt slices interleave at tuned points.
- main tiles (n,t): 8 DoubleRow MMs + 1 bf16 LoRA MM into one PSUM bank
  (opool bufs=6 rotation), evicted via ACT (scale+bias, fp16) into per-run
  staging tiles, flushed as one DMA per 4-column run.
- out-flushes for waves that overlap the in-stream are DEFERRED until the
  in-stream is fully queued (out transfers otherwise steal serial DMA slots
  from the critical x/w chunks); the final wave flushes per-tile from the
  then-idle SP queue to shorten the tail.
Engine usage: PE 89% busy; evictions on ACT; casts+masks on DVE; in-DMAs
issued from SP, mid-kernel out-DMAs from ACT (avoids SP head-of-line
blocking behind eviction-dependent waits).
"""

import numpy as np
import ml_dtypes

import concourse.mybir as mybir
import concourse.tile as tile
from concourse import bacc
from concourse.bass import ts
from concourse.bass_utils import run_bass_kernel_spmd

N_CORES = 8
B, S, D_IN, D_OUT = 4, 4096, 2048, 2048
N_TOK = B * S              # 16384
TOK = N_TOK // N_CORES     # 2048 tokens per core
A, R = 8, 16
AR = A * R                 # 128
P = 128
KC = D_IN // P             # 16 contraction chunks of 128
KP = KC // 2               # 8 DoubleRow k-pairs
NT = TOK // 512            # 4 token blocks of 512
NO = D_OUT // P            # 16 d_out columns of 128
WSCALE = 32.0              # fp8 pre-scale for W (and lbt); descaled on evict

BF16 = mybir.dt.bfloat16
F16 = mybir.dt.float16
F32 = mybir.dt.float32
F8 = mybir.dt.float8e4
DR = mybir.MatmulPerfMode.DoubleRow
IDENT = mybir.ActivationFunctionType.Identity

_cached_nc = None


def _build():
    nc = bacc.Bacc("TRN2", target_bir_lowering=False, debug=False)
    xhi = nc.dram_tensor("xhi", [D_IN, TOK], F8, kind="ExternalInput").ap()
    xlo = nc.dram_tensor("xlo", [D_IN, TOK], F8, kind="ExternalInput").ap()
    wf8 = nc.dram_tensor("wf8", [P, NO * KC * P], F8, kind="ExternalInput").ap()
    lathi = nc.dram_tensor("lathi", [P, KC * AR], F8, kind="ExternalInput").ap()
    latlo = nc.dram_tensor("latlo", [P, KC * AR], F8, kind="ExternalInput").ap()
    lbt = nc.dram_tensor("lbt", [AR, D_OUT], BF16, kind="ExternalInput").ap()
    selt = nc.dram_tensor("selt", [AR, TOK], BF16, kind="ExternalInput").ap()
    biasc = nc.dram_tensor("biasc", [P, NO], F32, kind="ExternalInput").ap()
    out = nc.dram_tensor("out", [D_OUT, TOK], F16, kind="ExternalOutput").ap()

    with tile.TileContext(nc) as tc:
        with (
            tc.tile_pool(name="const", bufs=1) as cpool,
            tc.tile_pool(name="work", bufs=4) as wpool,
            tc.tile_pool(name="psum_u", bufs=2, space="PSUM") as upool,
            tc.tile_pool(name="psum_o", bufs=6, space="PSUM") as opool,
        ):
            lathi_sb = cpool.tile([P, KC, AR], F8, tag="lathi")
            latlo_sb = cpool.tile([P, KC, AR], F8, tag="latlo")
            lbt_sb = cpool.tile([P, D_OUT], BF16, tag="lbt")
            selt_sb = cpool.tile([P, TOK], BF16, tag="selt")
            bias_sb = cpool.tile([P, NO], F32, tag="bias")
            xhi_sb = cpool.tile([P, KC, TOK], F8, tag="xhi")
            xlo_sb = cpool.tile([P, KC, TOK], F8, tag="xlo")
            wf8_sb = cpool.tile([P, NO * KC, P], F8, tag="wf8")

            # PE p-state warmup: dummy matmuls on memset tiles from t~0 so the
            # tensor engine is at full clock when the first real operand lands
            warm_a = cpool.tile([P, P], BF16, tag="warm_a")
            warm_b = cpool.tile([P, 512], BF16, tag="warm_b")
            nc.gpsimd.memset(warm_a[:], 0)
            nc.gpsimd.memset(warm_b[:], 0)
            warm_ps = upool.tile([P, 512], F32, tag="u", name="warm_ps")
            for i in range(9):
                nc.tensor.matmul(warm_ps[:], warm_a[:], warm_b[:],
                                 start=True, stop=True)

            u_m = [wpool.tile([P, 512], BF16, tag=f"um{t}", bufs=1, name=f"um{t}")
                   for t in range(NT)]

            def dma_x(src, dst, t, k0, k1):
                nc.sync.dma_start(
                    out=dst[:, k0:k1, ts(t, 512)],
                    in_=src[k0 * P:k1 * P, ts(t, 512)]
                    .rearrange("(k p) f -> p k f", p=P))

            def dma_selt(t):
                nc.sync.dma_start(out=selt_sb[:, ts(t, 512)],
                                  in_=selt[:, ts(t, 512)])

            def u_mms(t, kp, ua_ps, uy_ps, start):
                # u in fp8 DoubleRow, three scale-matched terms:
                #   ua = sum 32la_hi * x_hi            (scale 32)
                #   uy = sum 32la_hi * 64x_lo
                #      + sum 64la_lo * x_hi            (scale 2048)
                # combined at mask time as u = (ua + uy/64) / 32
                sl = (slice(None), slice(2 * kp, 2 * kp + 2))
                nc.tensor.matmul(ua_ps[:], lathi_sb[sl], xhi_sb[sl + (ts(t, 512),)],
                                 start=start, stop=(kp == KP - 1), perf_mode=DR)
                nc.tensor.matmul(uy_ps[:], lathi_sb[sl], xlo_sb[sl + (ts(t, 512),)],
                                 start=start, stop=False, perf_mode=DR)
                nc.tensor.matmul(uy_ps[:], latlo_sb[sl], xhi_sb[sl + (ts(t, 512),)],
                                 start=False, stop=(kp == KP - 1), perf_mode=DR)

            def u_mask(t, ua_ps, uy_ps):
                # hw: only one non-scalar input may read PSUM -> stage ua in
                # SBUF first (bf16 is plenty: ua holds 32*u)
                ua_sb = wpool.tile([P, 512], BF16, tag="uasb", name="ua_sb")
                nc.vector.tensor_copy(out=ua_sb[:], in_=ua_ps[:])
                utmp = wpool.tile([P, 512], BF16, tag="utmp", name="utmp")
                nc.vector.scalar_tensor_tensor(
                    out=utmp[:], in0=uy_ps[:], scalar=1.0 / 64, in1=ua_sb[:],
                    op0=mybir.AluOpType.mult, op1=mybir.AluOpType.add)
                nc.vector.tensor_mul(out=u_m[t][:], in0=utmp[:],
                                     in1=selt_sb[:, ts(t, 512)])

            def stream_xblock(t, wbatch=None, skip_h1=False, prefetch=None):
                # x hi chunks first (they gate the next waves' main tiles and
                # the ua/uy-hi matmuls), then the W-column batch for the
                # upcoming waves, then the lo chunks for the uy-lo term (only
                # needed by the mask at block end). prefetch emits the next
                # block's first xhi DMA between the xlo halves so the next
                # block's u-matmuls aren't starved at the boundary.
                ua_ps = upool.tile([P, 512], F32, tag="u", name=f"ua{t}")
                uy_ps = upool.tile([P, 512], F32, tag="u", name=f"uy{t}")
                sl512 = ts(t, 512)
                for h in range(2):
                    if not (h == 0 and skip_h1):
                        dma_x(xhi, xhi_sb, t, 8 * h, 8 * (h + 1))
                    for kp in range(4 * h, 4 * (h + 1)):
                        sl = (slice(None), slice(2 * kp, 2 * kp + 2))
                        nc.tensor.matmul(ua_ps[:], lathi_sb[sl],
                                         xhi_sb[sl + (sl512,)],
                                         start=(kp == 0), stop=(kp == KP - 1),
                                         perf_mode=DR)
                        nc.tensor.matmul(uy_ps[:], latlo_sb[sl],
                                         xhi_sb[sl + (sl512,)],
                                         start=(kp == 0), stop=False,
                                         perf_mode=DR)
                if wbatch is not None:
                    dma_wcols(wbatch)
                for h in range(2):
                    dma_x(xlo, xlo_sb, t, 8 * h, 8 * (h + 1))
                    if h == 0 and prefetch is not None:
                        prefetch()
                    for kp in range(4 * h, 4 * (h + 1)):
                        sl = (slice(None), slice(2 * kp, 2 * kp + 2))
                        nc.tensor.matmul(uy_ps[:], lathi_sb[sl],
                                         xlo_sb[sl + (sl512,)],
                                         start=False, stop=(kp == KP - 1),
                                         perf_mode=DR)
                dma_selt(t)
                u_mask(t, ua_ps, uy_ps)

            def dma_wcols(ns):
                n0, n1 = ns[0], ns[-1] + 1
                nc.sync.dma_start(
                    out=wf8_sb[:, n0 * KC:n1 * KC, :],
                    in_=wf8[:, n0 * KC * P:n1 * KC * P]
                    .rearrange("p (k c) -> p k c", c=P))

            deferred = []

            def flush_run(run, eng):
                t, n0, g, osb = run
                eng.dma_start(
                    out=out[n0 * P:(n0 + g) * P, ts(t, 512)]
                    .rearrange("(g p) f -> p g f", p=P),
                    in_=osb[:].rearrange("p (g f) -> p g f", g=g))

            def flush_deferred():
                for run in deferred:
                    flush_run(run, nc.sync)
                deferred.clear()

            def main_tiles(pairs, out_eng=None, defer=False):
                # pairs must be runs of contiguous n at fixed t; evictions
                # stage into one wide tile per run and flush with a single
                # DMA ([128, G, 512] SBUF -> [G*128, 512] DRAM rows), keeping
                # the out-path from fragmenting the input stream's DMA slots.
                # defer=True postpones the flush DMA until flush_deferred()
                # so out transfers don't steal DMA slots from the in-stream.
                runs = []
                for (t, n) in pairs:
                    if (runs and runs[-1][0] == t and len(runs[-1][2]) < 4
                            and runs[-1][1] + len(runs[-1][2]) == n):
                        runs[-1][2].append(n)
                    else:
                        runs.append([t, n, [n]])
                for t, n0, ns in runs:
                    g = len(ns)
                    osb = wpool.tile([P, g * 512], F16, name="osb",
                                     tag="osb" if g == 4 else "osb1", bufs=10 if g == 4 else 4)
                    for j, n in enumerate(ns):
                        o_ps = opool.tile([P, 512], F32, tag="o")
                        for kp in range(KP):
                            nc.tensor.matmul(
                                o_ps[:],
                                wf8_sb[:, n * KC + 2 * kp:n * KC + 2 * kp + 2, :],
                                xhi_sb[:, 2 * kp:2 * kp + 2, ts(t, 512)],
                                start=(kp == 0), stop=False, perf_mode=DR)
                        nc.tensor.matmul(o_ps[:], lbt_sb[:, ts(n, P)], u_m[t][:],
                                         start=False, stop=True)
                        nc.scalar.activation(osb[:, ts(j, 512)], o_ps[:], IDENT,
                                             bias=bias_sb[:, n:n + 1],
                                             scale=1.0 / WSCALE)
                    if defer:
                        deferred.append((t, n0, g, osb))
                    else:
                        flush_run((t, n0, g, osb),
                                  out_eng if out_eng is not None else nc.scalar)

            # interleave DMA stream, u path, and main tiles in operand-arrival
            # order (see module docstring timeline). Head is fine-grained so
            # the u-matmuls start as early as the DMA serialization allows.
            # t0 head: xhi at k-pair granularity with the u hi-terms chasing;
            # w0-3 mid-phase; then the (t0, n0-3) DR accumulations run while
            # xlo streams for the uy-lo term; B matmuls after the mask.
            dma_x(xhi, xhi_sb, 0, 0, 2)
            nc.sync.dma_start(out=lathi_sb[:].rearrange("p k a -> p (k a)"),
                              in_=lathi[:, :])
            nc.sync.dma_start(out=latlo_sb[:].rearrange("p k a -> p (k a)"),
                              in_=latlo[:, :])
            ua0 = upool.tile([P, 512], F32, tag="u", name="ua0")
            uy0 = upool.tile([P, 512], F32, tag="u", name="uy0")
            for kp in range(8):
                if kp > 0:
                    dma_x(xhi, xhi_sb, 0, 2 * kp, 2 * kp + 2)
                if kp == 2:
                    dma_wcols(range(0, 4))
                sl = (slice(None), slice(2 * kp, 2 * kp + 2))
                nc.tensor.matmul(ua0[:], lathi_sb[sl], xhi_sb[sl + (ts(0, 512),)],
                                 start=(kp == 0), stop=(kp == KP - 1),
                                 perf_mode=DR)
                nc.tensor.matmul(uy0[:], latlo_sb[sl], xhi_sb[sl + (ts(0, 512),)],
                                 start=(kp == 0), stop=False, perf_mode=DR)
                if kp in (2, 3):
                    # w0-3's transfer stalls the xhi chase here; keep the PE
                    # clock hot with filler matmuls
                    for i in range(4):
                        nc.tensor.matmul(warm_ps[:], warm_a[:], warm_b[:],
                                         start=True, stop=True)
            dma_x(xlo, xlo_sb, 0, 0, 8)
            dma_x(xlo, xlo_sb, 0, 8, 16)
            dma_selt(0)
            nc.sync.dma_start(out=bias_sb[:], in_=biasc[:, :])
            nc.sync.dma_start(out=lbt_sb[:], in_=lbt[:, :])
            ocm0 = []
            osb0 = wpool.tile([P, 4 * 512], F16, name="osb", tag="osb", bufs=10)
            for n in range(4):
                o_ps = opool.tile([P, 512], F32, tag="o", name="ocm0")
                for kp in range(KP):
                    nc.tensor.matmul(
                        o_ps[:],
                        wf8_sb[:, n * KC + 2 * kp:n * KC + 2 * kp + 2, :],
                        xhi_sb[:, 2 * kp:2 * kp + 2, ts(0, 512)],
                        start=(kp == 0), stop=False, perf_mode=DR)
                ocm0.append(o_ps)
            for h in range(2):
                for kp in range(4 * h, 4 * (h + 1)):
                    sl = (slice(None), slice(2 * kp, 2 * kp + 2))
                    nc.tensor.matmul(uy0[:], lathi_sb[sl],
                                     xlo_sb[sl + (ts(0, 512),)],
                                     start=False, stop=(kp == KP - 1),
                                     perf_mode=DR)
            u_mask(0, ua0, uy0)
            for j, (n, o_ps) in enumerate(zip(range(4), ocm0)):
                nc.tensor.matmul(o_ps[:], lbt_sb[:, ts(n, P)], u_m[0][:],
                                 start=False, stop=True)
                nc.scalar.activation(osb0[:, ts(j, 512)], o_ps[:], IDENT,
                                     bias=bias_sb[:, n:n + 1],
                                     scale=1.0 / WSCALE)
            deferred.append((0, 0, 4, osb0))
            stream_xblock(1, wbatch=range(4, 8))
            main_tiles([(1, n) for n in range(0, 4)], defer=True)
            main_tiles([(t, n) for t in (0, 1) for n in range(4, 8)], defer=True)
            stream_xblock(2, wbatch=range(8, 12))
            main_tiles([(2, n) for n in range(0, 8)], defer=True)
            main_tiles([(t, n) for t in (0, 1) for n in range(8, 12)], defer=True)
            stream_xblock(3, wbatch=range(12, 16))
            flush_deferred()
            main_tiles([(2, n) for n in range(8, 12)])
            main_tiles([(3, n) for n in range(0, 12)])
            main_tiles([(t, n) for t in (0, 1, 2) for n in range(12, 16)],
                       out_eng=nc.sync)
            for n in range(12, 16):
                main_tiles([(3, n)], out_eng=nc.sync)
    nc.compile()
    return nc


def _get_nc():
    global _cached_nc
    if _cached_nc is None:
        _cached_nc = _build()
    return _cached_nc


def _prep_shared(weight, bias, lora_a, lora_b, scaling):
    bf16 = ml_dtypes.bfloat16
    e4 = ml_dtypes.float8_e4m3
    wt = np.asarray(weight, np.float32).T * WSCALE          # [dk, dout]
    wf8_h = np.ascontiguousarray(
        wt.reshape(KC, P, NO, P).transpose(1, 2, 0, 3).reshape(P, NO * KC * P)
    ).astype(e4)
    la = np.asarray(lora_a, np.float32).reshape(AR, D_IN)   # [ar, dk]
    la32 = np.ascontiguousarray(
        la.reshape(AR, KC, P).transpose(2, 1, 0).reshape(P, KC * AR)) * WSCALE
    lathi_h = la32.astype(e4)
    latlo_h = ((la32 - lathi_h.astype(np.float32)) * 64.0).astype(e4)
    lb = np.asarray(lora_b, np.float32) * np.asarray(scaling, np.float32)[:, None, None]
    lbt_h = (np.ascontiguousarray(lb.transpose(0, 2, 1).reshape(AR, D_OUT))
             * WSCALE).astype(bf16)
    biasc_h = np.ascontiguousarray(
        np.asarray(bias, np.float32).reshape(NO, P).T)
    return wf8_h, lathi_h, latlo_h, lbt_h, biasc_h


def _prep_core(x2, mapping, c):
    bf16 = ml_dtypes.bfloat16
    e4 = ml_dtypes.float8_e4m3
    xs = np.ascontiguousarray(x2[c * TOK:(c + 1) * TOK].T)   # [dk, tok]
    xhi_h = xs.astype(e4)
    xlo_h = ((xs - xhi_h.astype(np.float32)) * 64.0).astype(e4)
    ms = mapping[c * TOK:(c + 1) * TOK]
    aid = np.arange(1, A + 1, dtype=np.int32)
    onehot = (ms[None, :] == aid[:, None]).astype(np.float32)    # [A, TOK]
    # fold the 1/WSCALE descale of the u psums into the mask
    selt_h = np.ascontiguousarray(
        np.repeat(onehot, R, axis=0) / WSCALE).astype(bf16)
    return xhi_h, xlo_h, selt_h


def kernel(x, lora_mapping, weight, bias, lora_a, lora_b, scaling):
    nc = _get_nc()
    wf8_h, lathi_h, latlo_h, lbt_h, biasc_h = _prep_shared(
        weight, bias, lora_a, lora_b, scaling)
    x2 = np.asarray(x, np.float32).reshape(N_TOK, D_IN)
    mapping = np.asarray(lora_mapping, np.int32)

    in_maps = []
    for c in range(N_CORES):
        xhi_h, xlo_h, selt_h = _prep_core(x2, mapping, c)
        in_maps.append({
            "xhi": xhi_h, "xlo": xlo_h, "wf8": wf8_h, "lathi": lathi_h,
            "latlo": latlo_h, "lbt": lbt_h, "selt": selt_h, "biasc": biasc_h,
        })

    res = run_bass_kernel_spmd(nc, in_maps, list(range(N_CORES)))
    outs = [np.asarray(res.results[c]["out"], np.float32).T
            for c in range(N_CORES)]
    return np.ascontiguousarray(np.concatenate(outs, axis=0)).reshape(B, S, D_OUT)
